# revision 13
# baseline (speedup 1.0000x reference)
"""Multi-head attention (ALiBi) Trainium2 kernel.

B=4, T=2048, C=1024, H=16 heads, D=64. 8 NeuronCores: core c handles
batch b=c//2 and head-group g=c%2 (8 heads, channel slice of 512).

Per-core dataflow (all matmuls float32r):
  phase 1: QT/KT = (Wq|Wk)^T x^T (+bias, Q scaled by 1/8), V = x Wv + bv
  phase 2: per (head, q-chunk): scoresT[k,q] = KT_chunk^T QT; ACT exp with
           per-partition ALiBi bias -> expT (f32r); attn@V via lhsT=[V|1]
           accumulating outT[65,512] (row 64 = softmax denominator);
           PE rank-1 broadcast of denom -> approx reciprocal -> DVE
           tensor_tensor normalize -> DMA attn^T to HBM; attn-out rows
           staged to DRAM scratch.
  phase 3: out_partial = attnOut^T^T Wo + bo/2, written [2048, 1024].

Host: shards/pre-arranges inputs, sums the two head-group partials per
batch, and transposes attn^T -> attn while gathering.
"""

import numpy as np

import concourse.bacc as bacc
import concourse.bass as bass
import concourse.mybir as mybir
import concourse.tile as tile
from concourse.bass_utils import run_bass_kernel_spmd

f32 = mybir.dt.float32
f32r = mybir.dt.float32r
AF = mybir.ActivationFunctionType

B, T, C, H = 4, 2048, 1024, 16
D = 64          # head dim
HL = 8          # heads per core
CH = 512        # channels per core
NCI = 8         # cin chunks (C/128)
NTC = 4         # t-chunks of 512 (phase 1)
NKC = 16        # k-chunks of 128
NQC = 4         # q-chunks of 512
QW = 512

_CACHE = {}


def _build_module():
    nc = bacc.Bacc("TRN2", target_bir_lowering=False, debug=False)

    xT_d = nc.dram_tensor("xT", [128, NCI, T], f32r, kind="ExternalInput")
    wq_d = nc.dram_tensor("wq", [128, NCI, CH], f32r, kind="ExternalInput")
    wk_d = nc.dram_tensor("wk", [128, NCI, CH], f32r, kind="ExternalInput")
    wv_d = nc.dram_tensor("wv", [128, NCI, CH], f32r, kind="ExternalInput")
    wo_d = nc.dram_tensor("wo", [128, 4, C], f32r, kind="ExternalInput")
    bq8_d = nc.dram_tensor("bq8", [128, 4], f32, kind="ExternalInput")
    bk_d = nc.dram_tensor("bk", [128, 4], f32, kind="ExternalInput")
    bv_d = nc.dram_tensor("bv", [CH], f32, kind="ExternalInput")
    bo2_d = nc.dram_tensor("bo2", [1, C], f32r, kind="ExternalInput")
    alibi_d = nc.dram_tensor("alibi", [128, HL, NKC], f32, kind="ExternalInput")
    vones_d = nc.dram_tensor("vones", [128, NKC, HL], f32r, kind="ExternalInput")
    onesr_d = nc.dram_tensor("onesr", [1, 128], f32r, kind="ExternalInput")

    attnT_o = nc.dram_tensor("attnT", [HL, T, T], f32, kind="ExternalOutput")
    out_o = nc.dram_tensor("outp", [T, C], f32, kind="ExternalOutput")

    aot_d = nc.dram_tensor("aot_scratch", [4, 128, T], f32r, kind="Internal")

    with tile.TileContext(nc) as tc:
        with tc.tile_pool(name="persist", bufs=1) as persist:
            qt = persist.tile([128, 4, T], f32r)     # [cout%128, cout//128, t]
            kt = persist.tile([128, 4, T], f32r)
            vsb = persist.tile([128, NKC, HL * 65], f32r)  # [t%128, kc, h*65+d]
            ones_row = persist.tile([1, 128], f32r)
            nc.sync.dma_start(out=ones_row, in_=onesr_d[:, :])
            nc.sync.dma_start(
                out=vsb[:].rearrange("p kc (h x) -> p kc h x", x=65)[:, :, :, 64:65],
                in_=vones_d[:, :, :, None],
            )

            # ---------------- phase 1: projections ----------------
            with tc.tile_pool(name="p1w", bufs=1) as p1w, \
                 tc.tile_pool(name="p1x", bufs=2) as p1x, \
                 tc.tile_pool(name="ps1", bufs=4, space="PSUM") as ps1:
                wq = p1w.tile([128, NCI, CH], f32r)
                wk = p1w.tile([128, NCI, CH], f32r)
                wv = p1w.tile([128, NCI, CH], f32r)
                bq8 = p1w.tile([128, 4], f32)
                bk = p1w.tile([128, 4], f32)
                bvb = p1w.tile([128, CH], f32)
                nc.sync.dma_start(out=wq, in_=wq_d[:, :, :])
                xs0 = p1x.tile([128, NCI, QW], f32r, tag="xslab")
                nc.sync.dma_start(out=xs0, in_=xT_d[:, :, 0:QW])
                nc.sync.dma_start(out=bq8, in_=bq8_d[:, :])
                nc.sync.dma_start(out=wk, in_=wk_d[:, :, :])
                nc.sync.dma_start(out=bk, in_=bk_d[:, :])
                nc.sync.dma_start(out=wv, in_=wv_d[:, :, :])
                nc.sync.dma_start(
                    out=bvb,
                    in_=bass.AP(tensor=bv_d.ap().tensor, offset=0,
                                ap=[[0, 128], [1, CH]]),
                )

                for ti in range(NTC):
                    if ti == 0:
                        xs = xs0
                    else:
                        xs = p1x.tile([128, NCI, QW], f32r, tag="xslab")
                        nc.sync.dma_start(out=xs, in_=xT_d[:, :, ti * QW:(ti + 1) * QW])
                    # QT / KT: out [cout 128, t 512]
                    for w_sb, b_sb, dst, scale in (
                        (wq, bq8, qt, 0.125),
                        (wk, bk, kt, 1.0),
                    ):
                        for cc in range(4):
                            pt = ps1.tile([128, QW], f32, tag="proj")
                            for ci in range(NCI):
                                nc.tensor.matmul(
                                    pt,
                                    lhsT=w_sb[:, ci, cc * 128:(cc + 1) * 128],
                                    rhs=xs[:, ci, :],
                                    start=(ci == 0), stop=(ci == NCI - 1),
                                )
                            nc.scalar.activation(
                                dst[:, cc, ti * QW:(ti + 1) * QW], pt,
                                AF.Identity, bias=b_sb[:, cc:cc + 1], scale=scale,
                            )
                    # V: out [t 128, c 512]
                    for ts in range(4):
                        kc = ti * 4 + ts
                        pt = ps1.tile([128, CH], f32, tag="proj")
                        for ci in range(NCI):
                            nc.tensor.matmul(
                                pt,
                                lhsT=xs[:, ci, ts * 128:(ts + 1) * 128],
                                rhs=wv[:, ci, :],
                                start=(ci == 0), stop=(ci == NCI - 1),
                            )
                        nc.vector.tensor_tensor(
                            vsb[:, kc].rearrange("p (h x) -> p h x", x=65)[:, :, 0:64],
                            pt[:].rearrange("p (h d) -> p h d", d=64),
                            bvb[:].rearrange("p (h d) -> p h d", d=64),
                            mybir.AluOpType.add,
                        )

            # ---------------- phase 2: attention ----------------
            with tc.tile_pool(name="p2c", bufs=1) as p2c, \
                 tc.tile_pool(name="p2e", bufs=2) as p2e, \
                 tc.tile_pool(name="p2w", bufs=2) as p2w, \
                 tc.tile_pool(name="p2s", bufs=3) as p2s, \
                 tc.tile_pool(name="ps2a", bufs=4, space="PSUM") as ps2a, \
                 tc.tile_pool(name="ps2b", bufs=2, space="PSUM") as ps2b:
                alibi = p2c.tile([128, HL, NKC], f32)
                nc.sync.dma_start(out=alibi, in_=alibi_d[:, :, :])

                for h in range(HL):
                    p0 = (h % 2) * 64
                    cb = h // 2
                    for qc in range(NQC):
                        q0 = qc * QW
                        expT = p2e.tile([128, NKC, QW], f32r, tag="expT")
                        outT = ps2b.tile([65, QW], f32, tag="outT")
                        for kc in range(NKC):
                            sc = ps2a.tile([128, QW], f32, tag="sc")
                            nc.tensor.matmul(
                                sc,
                                lhsT=kt[p0:p0 + 64, cb, kc * 128:(kc + 1) * 128],
                                rhs=qt[p0:p0 + 64, cb, q0:q0 + QW],
                                start=True, stop=True,
                            )
                            nc.scalar.activation(
                                expT[:, kc, :], sc, AF.Exp,
                                bias=alibi[:, h, kc:kc + 1], scale=1.0,
                            )
                            nc.tensor.matmul(
                                outT,
                                lhsT=vsb[:, kc, h * 65:(h + 1) * 65],
                                rhs=expT[:, kc, :],
                                start=(kc == 0), stop=(kc == NKC - 1),
                            )
                        colsum = p2w.tile([1, QW], f32r, tag="colsum")
                        nc.scalar.copy(colsum, outT[64:65, :])
                        bcast = ps2b.tile([128, QW], f32, tag="bcast")
                        nc.tensor.matmul(bcast, lhsT=ones_row, rhs=colsum,
                                         start=True, stop=True)
                        recip = p2w.tile([128, QW], f32, tag="recip")
                        nc.vector.reciprocal_approx_fast(out=recip[:], in_=bcast[:])
                        for m in range(4):
                            stage = p2s.tile([128, 4, QW], f32, tag="stage")
                            nc.vector.tensor_tensor(
                                stage[:],
                                expT[:, 4 * m:4 * m + 4, :].bitcast(f32),
                                recip[:].unsqueeze(1).to_broadcast([128, 4, QW]),
                                mybir.AluOpType.mult,
                            )
                            nc.sync.dma_start(
                                out=attnT_o[h, 4 * m * 128:(4 * m + 4) * 128,
                                            q0:q0 + QW]
                                .rearrange("(j p) q -> p j q", p=128),
                                in_=stage,
                            )
                        aotst = p2w.tile([64, QW], f32r, tag="aotst")
                        nc.vector.tensor_tensor(
                            aotst[:], outT[0:64, :], recip[0:64, :],
                            mybir.AluOpType.mult,
                        )
                        nc.sync.dma_start(
                            out=aot_d[cb, p0:p0 + 64, q0:q0 + QW], in_=aotst)

            # ---------------- phase 3: output projection ----------------
            with tc.tile_pool(name="p3", bufs=1) as p3, \
                 tc.tile_pool(name="p3o", bufs=2) as p3o, \
                 tc.tile_pool(name="ps3", bufs=4, space="PSUM") as ps3:
                wo = p3.tile([128, 4, C], f32r)
                aot = p3.tile([128, 4, T], f32r)
                bo2 = p3.tile([1, C], f32r)
                nc.sync.dma_start(out=wo, in_=wo_d[:, :, :])
                nc.sync.dma_start(out=aot, in_=aot_d[:, :, :].rearrange("c p t -> p c t"))
                nc.sync.dma_start(out=bo2, in_=bo2_d[:, :])
                for t16 in range(16):
                    outsb = p3o.tile([128, C], f32, tag="outsb")
                    for co in range(2):
                        pt = ps3.tile([128, 512], f32, tag="po")
                        for chc in range(4):
                            nc.tensor.matmul(
                                pt,
                                lhsT=aot[:, chc, t16 * 128:(t16 + 1) * 128],
                                rhs=wo[:, chc, co * 512:(co + 1) * 512],
                                start=(chc == 0), stop=False,
                            )
                        nc.tensor.matmul(
                            pt, lhsT=ones_row,
                            rhs=bo2[0:1, co * 512:(co + 1) * 512],
                            start=False, stop=True,
                        )
                        nc.vector.tensor_copy(outsb[:, co * 512:(co + 1) * 512], pt)
                    nc.sync.dma_start(
                        out=out_o[t16 * 128:(t16 + 1) * 128, :], in_=outsb)

    nc.compile()
    return nc


def _prep_core_inputs(c, x, Wq, bq, Wk, bk, Wv, bv, Wo, bo):
    b, g = c // 2, c % 2
    sl = slice(g * CH, (g + 1) * CH)
    xT = np.ascontiguousarray(
        x[b].T.reshape(NCI, 128, T).transpose(1, 0, 2))
    wq = np.ascontiguousarray(Wq[:, sl].reshape(NCI, 128, CH).transpose(1, 0, 2))
    wk = np.ascontiguousarray(Wk[:, sl].reshape(NCI, 128, CH).transpose(1, 0, 2))
    wv = np.ascontiguousarray(Wv[:, sl].reshape(NCI, 128, CH).transpose(1, 0, 2))
    wo = np.ascontiguousarray(Wo[sl, :].reshape(4, 128, C).transpose(1, 0, 2))
    bq8 = np.ascontiguousarray((bq[sl] / 8.0).reshape(4, 128).T)
    bk_a = np.ascontiguousarray(bk[sl].reshape(4, 128).T)
    bo2 = (bo / 2.0).reshape(1, C).astype(np.float32)

    slopes = (0.5 ** np.arange(1, H + 1, dtype=np.float64))[g * HL:(g + 1) * HL]
    k_idx = (np.arange(NKC)[None, :] * 128 + np.arange(128)[:, None]).astype(np.float64)
    alibi = (slopes[None, :, None] * (k_idx[:, None, :] - (T - 1.0))).astype(np.float32)

    return {
        "xT": xT, "wq": wq, "wk": wk, "wv": wv, "wo": wo,
        "bq8": bq8.astype(np.float32), "bk": bk_a.astype(np.float32),
        "bv": bv[sl].astype(np.float32), "bo2": bo2,
        "alibi": alibi,
        "vones": np.ones((128, NKC, HL), np.float32),
        "onesr": np.ones((1, 128), np.float32),
    }


def kernel(x, Wq, bq, Wk, bk, Wv, bv, Wo, bo, **run_kwargs):
    x = np.asarray(x, np.float32)
    Wq, bq = np.asarray(Wq, np.float32), np.asarray(bq, np.float32)
    Wk, bk = np.asarray(Wk, np.float32), np.asarray(bk, np.float32)
    Wv, bv = np.asarray(Wv, np.float32), np.asarray(bv, np.float32)
    Wo, bo = np.asarray(Wo, np.float32), np.asarray(bo, np.float32)

    if "nc" not in _CACHE:
        _CACHE["nc"] = _build_module()
    nc = _CACHE["nc"]

    in_maps = [_prep_core_inputs(c, x, Wq, bq, Wk, bk, Wv, bv, Wo, bo)
               for c in range(8)]
    res = run_bass_kernel_spmd(nc, in_maps, core_ids=list(range(8)), **run_kwargs)
    _CACHE["last_results"] = res

    out = np.empty((B, T, C), np.float32)
    attn = np.empty((B, H, T, T), np.float32)
    for b in range(B):
        r0, r1 = res.results[2 * b], res.results[2 * b + 1]
        out[b] = r0["outp"] + r1["outp"]
        attn[b, 0:HL] = r0["attnT"].transpose(0, 2, 1)
        attn[b, HL:H] = r1["attnT"].transpose(0, 2, 1)
    return out, attn
